# revision 1
# baseline (speedup 1.0000x reference)
"""Trainium2 Bass kernel for a 3-layer GCN encoder over two graphs (x, y).

Dense-adjacency formulation:
  GCNConv(h) = D^-1/2 (A+I) D^-1/2 (h @ W) + b
  With Acnt the self-loop-augmented adjacency-count matrix and dinv = deg^-1/2:
      Hhat_1   = dinv * x                   (host, shipped bf16)
      P_l      = Acnt @ Hhat_l              (PE matmul, dominant cost)
      S_l      = dinv * P_l                 (dst-side norm)
      z_l      = S_l @ W_l + b_l            (PE matmul; bias via rank-1 matmul)
      Hhat_l+1 = dinv * relu(z_l)           (src-side norm of next layer)
  Output layer: out = S_3 @ W_3 + b_3.

Sharding: all 8 cores form one replica group; each core owns a 1280-row
(1250 real) dst shard of BOTH graphs. Acnt^T is streamed from HBM as bf16
(exact small-integer counts); Hhat for both graphs is SBUF-resident and
replicated with a single 8-rank AllGather per hidden layer.

Node ids are renumbered into a padded space of 10240 = 8*1280 so all tiles
are 128-multiples and the AllGather output is directly the packed SBUF
image of Hhat.
"""

import numpy as np
import ml_dtypes

import concourse.bass as bass
import concourse.tile as tile
from concourse import bacc, mybir
import concourse.bass_utils as bass_utils
from concourse.masks import make_identity

BF16 = ml_dtypes.bfloat16

P = 128          # partitions / tile edge
NC = 8           # cores
N_NODES = 10000
SHARD = 1250     # real nodes per core (per graph)
SHP = 1280       # padded nodes per core
NPAD = NC * SHP  # 10240
KT = NPAD // P   # 80 k-tiles over src nodes
MT = SHP // P    # 10 m-tiles per graph per core
F = 256          # in/hidden feature width
FO = 128         # output feature width

_NC_CACHE = {}


# ----------------------------------------------------------------------------
# Host-side graph preprocessing (index/static work only)
# ----------------------------------------------------------------------------

def _pad_ids(n):
    return (n // SHARD) * SHP + (n % SHARD)


def _prep_graph(x, edge_index, Ws, bs):
    """Returns (per-core list of 8 slab tensors, h1_img, w_imgs, b_rows, dinv_pad)."""
    src = edge_index[0].astype(np.int64)
    dst = edge_index[1].astype(np.int64)
    loop = np.arange(N_NODES, dtype=np.int64)
    src = np.concatenate([src, loop])
    dst = np.concatenate([dst, loop])
    sp = _pad_ids(src)
    dp = _pad_ids(dst)

    deg = np.zeros(NPAD, np.float32)
    np.add.at(deg, dp, np.float32(1.0))
    dinv = np.zeros(NPAD, np.float32)
    nz = deg > 0
    dinv[nz] = 1.0 / np.sqrt(deg[nz])

    at = np.zeros((NPAD, NPAD), np.float32)   # [src, dst] = A^T counts
    np.add.at(at, (sp, dp), np.float32(1.0))

    h1 = np.zeros((NPAD, F), np.float32)
    h1[_pad_ids(loop)] = x * dinv[_pad_ids(loop)][:, None]
    h1_img = np.ascontiguousarray(
        h1.reshape(KT, P, F).transpose(1, 0, 2).reshape(P, KT * F)
    ).astype(BF16)

    def w_img(W, fo):
        kf = W.shape[0] // P
        return np.ascontiguousarray(
            W.reshape(kf, P, fo).transpose(1, 0, 2).reshape(P, kf * fo)
        ).astype(BF16)

    slabs = []
    for g in range(NC):
        shard = at[:, g * SHP:(g + 1) * SHP]  # [NPAD src, SHP dst]
        slab = np.ascontiguousarray(
            shard.reshape(KT, P, MT, P).transpose(2, 1, 0, 3).reshape(MT, P, KT * P)
        ).astype(BF16)
        slabs.append(slab)
    w_imgs = [w_img(Ws[0], F), w_img(Ws[1], F), w_img(Ws[2], FO)]
    b_rows = [bs[0].reshape(1, F).astype(BF16),
              bs[1].reshape(1, F).astype(BF16),
              bs[2].reshape(1, FO).astype(BF16)]
    return slabs, h1_img, w_imgs, b_rows, dinv


def prep_in_maps(x, x_edge_index, y, y_edge_index,
                 W1x, b1x, W2x, b2x, W3x, b3x,
                 W1y, b1y, W2y, b2y, W3y, b3y):
    sx, h1x, wx, bx, dx = _prep_graph(
        np.asarray(x, np.float32), np.asarray(x_edge_index),
        (np.asarray(W1x), np.asarray(W2x), np.asarray(W3x)),
        (np.asarray(b1x), np.asarray(b2x), np.asarray(b3x)))
    sy, h1y, wy, by, dy = _prep_graph(
        np.asarray(y, np.float32), np.asarray(y_edge_index),
        (np.asarray(W1y), np.asarray(W2y), np.asarray(W3y)),
        (np.asarray(b1y), np.asarray(b2y), np.asarray(b3y)))
    maps = []
    for c in range(NC):
        dvx = dx[c * SHP:(c + 1) * SHP].reshape(MT, P).T   # [P, MT]
        dvy = dy[c * SHP:(c + 1) * SHP].reshape(MT, P).T
        maps.append({
            "at": np.stack([sx[c], sy[c]]),   # [2, MT, P, KT*P]
            "hx": h1x, "hy": h1y,
            "w0": wx[0], "w1": wx[1], "w2": wx[2],
            "w3": wy[0], "w4": wy[1], "w5": wy[2],
            "b0": bx[0], "b1": bx[1], "b2": bx[2],
            "b3": by[0], "b4": by[1], "b5": by[2],
            "dinv": np.ascontiguousarray(
                np.concatenate([dvx, dvy], axis=1)).astype(np.float32),  # [P, 2*MT]
        })
    return maps


def _unshard(z_imgs, graph):
    """8 per-core [P, 2*MT*FO] images -> [N_NODES, FO] for graph 0(x)/1(y)."""
    rows = []
    for z in z_imgs:
        zi = z.reshape(P, 2 * MT, FO)[:, graph * MT:(graph + 1) * MT, :]
        r = zi.transpose(1, 0, 2).reshape(SHP, FO)
        rows.append(r[:SHARD])
    return np.concatenate(rows, axis=0)


# ----------------------------------------------------------------------------
# Device kernel
# ----------------------------------------------------------------------------

def _build_nc():
    if "nc" in _NC_CACHE:
        return _NC_CACHE["nc"]
    nc = bacc.Bacc("TRN2", target_bir_lowering=False, debug=False, num_devices=NC)
    dt = mybir.dt

    at = nc.dram_tensor("at", [2, MT, P, KT * P], dt.bfloat16, kind="ExternalInput").ap()
    hx = nc.dram_tensor("hx", [P, KT * F], dt.bfloat16, kind="ExternalInput").ap()
    hy = nc.dram_tensor("hy", [P, KT * F], dt.bfloat16, kind="ExternalInput").ap()
    w_ap = [nc.dram_tensor(f"w{i}", [P, 2 * (FO if i % 3 == 2 else F)], dt.bfloat16,
                           kind="ExternalInput").ap() for i in range(6)]
    b_ap = [nc.dram_tensor(f"b{i}", [1, FO if i % 3 == 2 else F], dt.bfloat16,
                           kind="ExternalInput").ap() for i in range(6)]
    dinv = nc.dram_tensor("dinv", [P, 2 * MT], dt.float32, kind="ExternalInput").ap()
    zout = nc.dram_tensor("z", [P, 2 * MT * FO], dt.float32, kind="ExternalOutput").ap()

    groups = [list(range(NC))]

    with tile.TileContext(nc) as tc:
        with (
            tc.tile_pool(name="persist", bufs=1) as pers,
            tc.tile_pool(name="aslab", bufs=3) as apool,
            tc.tile_pool(name="work", bufs=4) as wk,
            tc.tile_pool(name="pagg", bufs=2, space="PSUM") as pagg,
            tc.tile_pool(name="ptr", bufs=2, space="PSUM") as ptr,
            tc.tile_pool(name="pg", bufs=2, space="PSUM") as pg,
            tc.tile_pool(name="dram", bufs=1, space="DRAM") as dp,
        ):
            Hg = [pers.tile([P, KT * F], dt.bfloat16, name="Hx"),
                  pers.tile([P, KT * F], dt.bfloat16, name="Hy")]
            Hown = pers.tile([P, 2 * MT * F], dt.bfloat16)
            Zsb = pers.tile([P, 2 * MT * FO], dt.float32)
            Wt = [pers.tile([P, 2 * (FO if i % 3 == 2 else F)], dt.bfloat16,
                            name=f"wt{i}") for i in range(6)]
            Bt = [pers.tile([1, FO if i % 3 == 2 else F], dt.bfloat16, name=f"bt{i}")
                  for i in range(6)]
            Dv = pers.tile([P, 2 * MT], dt.float32)
            ident = pers.tile([P, P], dt.bfloat16)
            ones = pers.tile([1, P], dt.bfloat16)

            make_identity(nc, ident[:])
            nc.gpsimd.memset(ones[:], 1.0)
            # chunked initial H loads so layer-1 matmuls can start early
            CH = KT * F // 4
            for r in range(4):
                nc.sync.dma_start(Hg[0][:, r * CH:(r + 1) * CH],
                                  hx[:, r * CH:(r + 1) * CH])
            for r in range(4):
                nc.sync.dma_start(Hg[1][:, r * CH:(r + 1) * CH],
                                  hy[:, r * CH:(r + 1) * CH])
            for i in range(6):
                nc.sync.dma_start(Wt[i][:], w_ap[i])
                nc.sync.dma_start(Bt[i][:], b_ap[i])
            nc.sync.dma_start(Dv[:], dinv)

            for layer in range(3):
                fo = FO if layer == 2 else F
                for g in range(2):
                    H = Hg[g]
                    Wl = Wt[3 * g + layer]
                    Bl = Bt[3 * g + layer]
                    for m in range(MT):
                        gm = g * MT + m
                        a_slab = apool.tile([P, KT * P], dt.bfloat16, tag="aslab")
                        # scalar-engine HWDGE queue: keeps A-slab streaming off
                        # the sync queue that carries H/W/B and AG reloads
                        nc.scalar.dma_start(a_slab[:], at[g, m])
                        pP = pagg.tile([P, F], dt.float32, tag="agg")
                        for k in range(KT):
                            nc.tensor.matmul(
                                pP[:],
                                lhsT=a_slab[:, k * P:(k + 1) * P],
                                rhs=H[:, k * F:(k + 1) * F],
                                start=(k == 0),
                                stop=(k == KT - 1),
                            )
                        S = wk.tile([P, F], dt.bfloat16, tag="S")
                        nc.vector.tensor_scalar_mul(S[:], pP[:], Dv[:, gm:gm + 1])
                        gps = pg.tile([P, fo], dt.float32, tag="g")
                        for kf in range(2):
                            pT = ptr.tile([P, P], dt.bfloat16, tag="tr")
                            nc.tensor.transpose(
                                pT[:], S[:, kf * P:(kf + 1) * P], ident[:]
                            )
                            STk = wk.tile([P, P], dt.bfloat16, tag="ST")
                            nc.vector.tensor_copy(STk[:], pT[:])
                            nc.tensor.matmul(
                                gps[:],
                                lhsT=STk[:],
                                rhs=Wl[:, kf * fo:(kf + 1) * fo],
                                start=(kf == 0),
                                stop=False,
                            )
                        nc.tensor.matmul(
                            gps[:],
                            lhsT=ones[:1, :],
                            rhs=Bl[:1, :fo],
                            start=False,
                            stop=True,
                        )
                        if layer < 2:
                            nc.scalar.activation(
                                Hown[:, gm * F:(gm + 1) * F],
                                gps[:],
                                mybir.ActivationFunctionType.Relu,
                                scale=Dv[:, gm:gm + 1],
                            )
                        else:
                            nc.vector.tensor_copy(
                                Zsb[:, gm * FO:(gm + 1) * FO], gps[:]
                            )
                    if layer < 2:
                        # Two half-AllGathers per graph, fired after m=4 and
                        # m=9 (emitted here, after the full m-loop, but each
                        # depends only on its 5 Hown tiles so Tile lets the
                        # first half fly mid-loop). Each half's latency hides
                        # under the remaining compute of this graph and the
                        # other graph's m-loop.
                        W2F = MT * F        # 2560 cols per rank in H
                        HW2 = W2F // 2      # 1280 cols per half
                        for half in range(2):
                            agin = dp.tile([P, HW2], dt.bfloat16,
                                           tag=f"agin{layer}{g}{half}")
                            agout = dp.tile([NC * P, HW2], dt.bfloat16,
                                            tag=f"agout{layer}{g}{half}")
                            nc.sync.dma_start(
                                agin[:],
                                Hown[:, g * W2F + half * HW2:
                                     g * W2F + (half + 1) * HW2])
                            nc.gpsimd.collective_compute(
                                "AllGather",
                                mybir.AluOpType.bypass,
                                replica_groups=groups,
                                ins=[agin[:].opt()],
                                outs=[agout[:].opt()],
                            )
                            for r in range(NC):
                                # gpsimd queue: reloads must not delay the
                                # next collective's input DMA on the sync ring
                                nc.gpsimd.dma_start(
                                    Hg[g][:, r * W2F + half * HW2:
                                          r * W2F + (half + 1) * HW2],
                                    agout[r * P:(r + 1) * P, :],
                                )
            nc.sync.dma_start(zout, Zsb[:])
    nc.compile()
    _NC_CACHE["nc"] = nc
    return nc


# ----------------------------------------------------------------------------
# Entry point
# ----------------------------------------------------------------------------

def kernel(x, x_edge_index, y, y_edge_index,
           W1x, b1x, W2x, b2x, W3x, b3x,
           W1y, b1y, W2y, b2y, W3y, b3y,
           _trace=False, _trace_cores=None):
    in_maps = prep_in_maps(x, x_edge_index, y, y_edge_index,
                           W1x, b1x, W2x, b2x, W3x, b3x,
                           W1y, b1y, W2y, b2y, W3y, b3y)
    nc = _build_nc()
    kw = {}
    if _trace:
        kw = dict(trace=True, trace_cores=_trace_cores or [0])
    res = bass_utils.run_bass_kernel_spmd(
        nc, in_maps, core_ids=list(range(NC)), **kw
    )
    z = [res.results[c]["z"] for c in range(NC)]
    out_x = _unshard(z, 0)
    out_y = _unshard(z, 1)
    if _trace:
        kernel._last_result = res
    return out_x, out_y



# revision 3
# speedup vs baseline: 1.2632x; 1.2632x over previous
"""Trainium2 Bass kernel for a 3-layer GCN encoder over two graphs (x, y).

Dense-adjacency formulation:
  GCNConv(h) = D^-1/2 (A+I) D^-1/2 (h @ W) + b
  With Acnt the self-loop-augmented adjacency-count matrix and dinv = deg^-1/2:
      Hhat_1   = dinv * x                   (host, shipped bf16)
      P_l      = Acnt @ Hhat_l              (PE matmul, dominant cost)
      S_l      = dinv * P_l                 (dst-side norm)
      z_l      = S_l @ W_l + b_l            (PE matmul; bias via rank-1 matmul)
      Hhat_l+1 = dinv * relu(z_l)           (src-side norm of next layer)
  Layer 3 is transform-first: U = Hhat_3 @ W3 is computed on own nodes
  (width 128, not 256), AllGathered, and aggregated as
      out = dinv * (Acnt @ U) + b3
  which halves both the layer-3 aggregation matmul work and the final
  AllGather payload.

Acnt^T is streamed from HBM as fp8e4 (small integer counts, exact).

Sharding: all 8 cores form one replica group; each core owns a 1280-row
dst shard of BOTH graphs (core 7: 1040 real + 240 pad). Node ids are used
identically (no renumbering); src k-tiles cover rows 0..10111 (KT=79),
the all-pad tail tile is skipped.
"""

import numpy as np
import ml_dtypes

import concourse.bass as bass
import concourse.tile as tile
from concourse import bacc, mybir
import concourse.bass_utils as bass_utils
from concourse.masks import make_identity

BF16 = ml_dtypes.bfloat16
FP8 = ml_dtypes.float8_e4m3

P = 128          # partitions / tile edge
NC = 8           # cores
N_NODES = 10000
SHP = 1280       # padded nodes per core (per graph); core 7 has 1040 real
NPAD = NC * SHP  # 10240
KT = 79          # src k-tiles (cover rows 0..10111; tail tile all-pad, skipped)
KTG = 80         # k-tiles in the gathered H/U images (NC*MT; tail never read)
MT = SHP // P    # 10 m-tiles per graph per core
F = 256          # in/hidden feature width
FO = 128         # output feature width

_NC_CACHE = {}


# ----------------------------------------------------------------------------
# Host-side graph preprocessing (index/static work only)
# ----------------------------------------------------------------------------

def _prep_graph(x, edge_index, Ws, bs):
    src = edge_index[0].astype(np.int64)
    dst = edge_index[1].astype(np.int64)
    loop = np.arange(N_NODES, dtype=np.int64)
    src = np.concatenate([src, loop])
    dst = np.concatenate([dst, loop])

    deg = np.zeros(NPAD, np.float32)
    np.add.at(deg, dst, np.float32(1.0))
    dinv = np.zeros(NPAD, np.float32)
    nz = deg > 0
    dinv[nz] = 1.0 / np.sqrt(deg[nz])

    at = np.zeros((KT * P, NPAD), np.float32)   # [src, dst] = A^T counts
    np.add.at(at, (src, dst), np.float32(1.0))

    h1 = np.zeros((KTG * P, F), np.float32)
    h1[:N_NODES] = x * dinv[:N_NODES, None]
    h1_img = np.ascontiguousarray(
        h1.reshape(KTG, P, F).transpose(1, 0, 2).reshape(P, KTG * F)
    ).astype(BF16)

    def w_img(W, fo):
        kf = W.shape[0] // P
        return np.ascontiguousarray(
            W.reshape(kf, P, fo).transpose(1, 0, 2).reshape(P, kf * fo)
        ).astype(BF16)

    slabs = []
    for g in range(NC):
        shard = at[:, g * SHP:(g + 1) * SHP]  # [KT*P src, SHP dst]
        slab = np.ascontiguousarray(
            shard.reshape(KT, P, MT, P).transpose(2, 1, 0, 3).reshape(MT, P, KT * P)
        ).astype(FP8)
        slabs.append(slab)
    w_imgs = [w_img(Ws[0], F), w_img(Ws[1], F), w_img(Ws[2], FO)]
    b_rows = [bs[0].reshape(1, F).astype(BF16),
              bs[1].reshape(1, F).astype(BF16)]
    b3mat = np.ascontiguousarray(
        np.tile(bs[2].reshape(1, FO), (P, 1))).astype(np.float32)
    return slabs, h1_img, w_imgs, b_rows, b3mat, dinv


def prep_in_maps(x, x_edge_index, y, y_edge_index,
                 W1x, b1x, W2x, b2x, W3x, b3x,
                 W1y, b1y, W2y, b2y, W3y, b3y):
    sx, h1x, wx, bx, b3x_m, dx = _prep_graph(
        np.asarray(x, np.float32), np.asarray(x_edge_index),
        (np.asarray(W1x), np.asarray(W2x), np.asarray(W3x)),
        (np.asarray(b1x), np.asarray(b2x), np.asarray(b3x)))
    sy, h1y, wy, by, b3y_m, dy = _prep_graph(
        np.asarray(y, np.float32), np.asarray(y_edge_index),
        (np.asarray(W1y), np.asarray(W2y), np.asarray(W3y)),
        (np.asarray(b1y), np.asarray(b2y), np.asarray(b3y)))
    maps = []
    for c in range(NC):
        dvx = dx[c * SHP:(c + 1) * SHP].reshape(MT, P).T   # [P, MT]
        dvy = dy[c * SHP:(c + 1) * SHP].reshape(MT, P).T
        maps.append({
            "at": np.stack([sx[c], sy[c]]),   # [2, MT, P, KT*P] fp8
            "hx": h1x, "hy": h1y,
            "w0": wx[0], "w1": wx[1], "w2": wx[2],
            "w3": wy[0], "w4": wy[1], "w5": wy[2],
            "b0": bx[0], "b1": bx[1],
            "b3": by[0], "b4": by[1],
            "b3mx": b3x_m, "b3my": b3y_m,
            "dinv": np.ascontiguousarray(
                np.concatenate([dvx, dvy], axis=1)).astype(np.float32),  # [P, 2*MT]
        })
    return maps


def _unshard(z_imgs, graph):
    """8 per-core [P, 2*MT*FO] images -> [N_NODES, FO] for graph 0(x)/1(y)."""
    rows = []
    for c, z in enumerate(z_imgs):
        zi = z.reshape(P, 2 * MT, FO)[:, graph * MT:(graph + 1) * MT, :]
        r = zi.transpose(1, 0, 2).reshape(SHP, FO)
        rows.append(r[:min(SHP, N_NODES - c * SHP)])
    return np.concatenate(rows, axis=0)


# ----------------------------------------------------------------------------
# Device kernel
# ----------------------------------------------------------------------------

def _build_nc():
    if "nc" in _NC_CACHE:
        return _NC_CACHE["nc"]
    nc = bacc.Bacc("TRN2", target_bir_lowering=False, debug=False, num_devices=NC)
    dt = mybir.dt

    at = nc.dram_tensor("at", [2, MT, P, KT * P], dt.float8e4, kind="ExternalInput").ap()
    hx = nc.dram_tensor("hx", [P, KTG * F], dt.bfloat16, kind="ExternalInput").ap()
    hy = nc.dram_tensor("hy", [P, KTG * F], dt.bfloat16, kind="ExternalInput").ap()
    w_ap = [nc.dram_tensor(f"w{i}", [P, 2 * (FO if i % 3 == 2 else F)], dt.bfloat16,
                           kind="ExternalInput").ap() for i in range(6)]
    b_ap = {i: nc.dram_tensor(f"b{i}", [1, F], dt.bfloat16,
                              kind="ExternalInput").ap() for i in (0, 1, 3, 4)}
    b3x_ap = nc.dram_tensor("b3mx", [P, FO], dt.float32, kind="ExternalInput").ap()
    b3y_ap = nc.dram_tensor("b3my", [P, FO], dt.float32, kind="ExternalInput").ap()
    dinv = nc.dram_tensor("dinv", [P, 2 * MT], dt.float32, kind="ExternalInput").ap()
    zout = nc.dram_tensor("z", [P, 2 * MT * FO], dt.float32, kind="ExternalOutput").ap()

    groups = [list(range(NC))]

    with tile.TileContext(nc) as tc:
        with (
            tc.tile_pool(name="persist", bufs=1) as pers,
            tc.tile_pool(name="aslab", bufs=3) as apool,
            tc.tile_pool(name="work", bufs=4) as wk,
            tc.tile_pool(name="pagg", bufs=2, space="PSUM") as pagg,
            tc.tile_pool(name="ptr", bufs=2, space="PSUM") as ptr,
            tc.tile_pool(name="pg", bufs=2, space="PSUM") as pg,
            tc.tile_pool(name="dram", bufs=1, space="DRAM") as dp,
        ):
            Hg = [pers.tile([P, KTG * F], dt.bfloat16, name="Hx"),
                  pers.tile([P, KTG * F], dt.bfloat16, name="Hy")]
            Ug = [pers.tile([P, NC * MT * FO], dt.bfloat16, name="Ux"),
                  pers.tile([P, NC * MT * FO], dt.bfloat16, name="Uy")]
            Hown = pers.tile([P, 2 * MT * F], dt.bfloat16)
            Uown = pers.tile([P, 2 * MT * FO], dt.bfloat16)
            Zsb = pers.tile([P, 2 * MT * FO], dt.float32)
            Wt = [pers.tile([P, 2 * (FO if i % 3 == 2 else F)], dt.bfloat16,
                            name=f"wt{i}") for i in range(6)]
            Bt = {i: pers.tile([1, F], dt.bfloat16, name=f"bt{i}")
                  for i in (0, 1, 3, 4)}
            B3 = [pers.tile([P, FO], dt.float32, name="b3x"),
                  pers.tile([P, FO], dt.float32, name="b3y")]
            Dv = pers.tile([P, 2 * MT], dt.float32)
            ident = pers.tile([P, P], dt.bfloat16)
            ones = pers.tile([1, P], dt.bfloat16)

            make_identity(nc, ident[:])
            nc.gpsimd.memset(ones[:], 1.0)
            # chunked initial H loads so layer-1 matmuls can start early
            nkt = KTG * F
            CH = nkt // 4
            bnds = [0, CH, 2 * CH, 3 * CH, nkt]
            for r in range(4):
                nc.sync.dma_start(Hg[0][:, bnds[r]:bnds[r + 1]],
                                  hx[:, bnds[r]:bnds[r + 1]])
            for r in range(4):
                nc.sync.dma_start(Hg[1][:, bnds[r]:bnds[r + 1]],
                                  hy[:, bnds[r]:bnds[r + 1]])
            for i in range(6):
                nc.sync.dma_start(Wt[i][:], w_ap[i])
            for i in (0, 1, 3, 4):
                nc.sync.dma_start(Bt[i][:], b_ap[i])
            nc.sync.dma_start(B3[0][:], b3x_ap)
            nc.sync.dma_start(B3[1][:], b3y_ap)
            nc.sync.dma_start(Dv[:], dinv)

            for layer in range(3):
                for g in range(2):
                    Wl = Wt[3 * g + layer] if layer < 2 else None
                    W3l = Wt[3 * g + 2]
                    Bl = Bt[3 * g + layer] if layer < 2 else None
                    for m in range(MT):
                        gm = g * MT + m
                        a_slab = apool.tile([P, KT * P], dt.float8e4, tag="aslab")
                        # scalar-engine HWDGE queue: keeps A-slab streaming off
                        # the sync queue that carries H/W/B and AG reloads
                        nc.scalar.dma_start(a_slab[:], at[g, m])
                        if layer < 2:
                            H = Hg[g]
                            pP = pagg.tile([P, F], dt.float32, tag="agg")
                            for k in range(KT):
                                nc.tensor.matmul(
                                    pP[:],
                                    lhsT=a_slab[:, k * P:(k + 1) * P],
                                    rhs=H[:, k * F:(k + 1) * F],
                                    start=(k == 0),
                                    stop=(k == KT - 1),
                                )
                            S = wk.tile([P, F], dt.bfloat16, tag="S")
                            nc.vector.tensor_scalar_mul(S[:], pP[:], Dv[:, gm:gm + 1])
                            gps = pg.tile([P, F], dt.float32, tag="g")
                            for kf in range(2):
                                pT = ptr.tile([P, P], dt.bfloat16, tag="tr")
                                nc.tensor.transpose(
                                    pT[:], S[:, kf * P:(kf + 1) * P], ident[:]
                                )
                                STk = wk.tile([P, P], dt.bfloat16, tag="ST")
                                nc.vector.tensor_copy(STk[:], pT[:])
                                nc.tensor.matmul(
                                    gps[:],
                                    lhsT=STk[:],
                                    rhs=Wl[:, kf * F:(kf + 1) * F],
                                    start=(kf == 0),
                                    stop=False,
                                )
                            nc.tensor.matmul(
                                gps[:],
                                lhsT=ones[:1, :],
                                rhs=Bl[:1, :F],
                                start=False,
                                stop=True,
                            )
                            # Hhat_{l+1} = dinv * relu(z) for own nodes
                            nc.scalar.activation(
                                Hown[:, gm * F:(gm + 1) * F],
                                gps[:],
                                mybir.ActivationFunctionType.Relu,
                                scale=Dv[:, gm:gm + 1],
                            )
                            if layer == 1:
                                # U = Hhat_3 @ W3 for own nodes (transform
                                # first: halves L3 aggregation width + AG)
                                pU = pg.tile([P, FO], dt.float32, tag="u")
                                for kf in range(2):
                                    pT = ptr.tile([P, P], dt.bfloat16, tag="tr")
                                    nc.tensor.transpose(
                                        pT[:],
                                        Hown[:, gm * F + kf * P:
                                             gm * F + (kf + 1) * P],
                                        ident[:],
                                    )
                                    HTk = wk.tile([P, P], dt.bfloat16, tag="HT")
                                    nc.vector.tensor_copy(HTk[:], pT[:])
                                    nc.tensor.matmul(
                                        pU[:],
                                        lhsT=HTk[:],
                                        rhs=W3l[:, kf * FO:(kf + 1) * FO],
                                        start=(kf == 0),
                                        stop=(kf == 1),
                                    )
                                nc.vector.tensor_copy(
                                    Uown[:, gm * FO:(gm + 1) * FO], pU[:]
                                )
                        else:
                            U = Ug[g]
                            pP = pagg.tile([P, FO], dt.float32, tag="agg")
                            for k in range(KT):
                                nc.tensor.matmul(
                                    pP[:],
                                    lhsT=a_slab[:, k * P:(k + 1) * P],
                                    rhs=U[:, k * FO:(k + 1) * FO],
                                    start=(k == 0),
                                    stop=(k == KT - 1),
                                )
                            # out = dinv * P3 + b3
                            S3 = wk.tile([P, FO], dt.float32, tag="S3")
                            nc.vector.tensor_scalar_mul(S3[:], pP[:], Dv[:, gm:gm + 1])
                            nc.vector.tensor_tensor(
                                Zsb[:, gm * FO:(gm + 1) * FO], S3[:], B3[g][:],
                                mybir.AluOpType.add,
                            )
                    if layer == 0:
                        # AllGather Hhat_2: two half-AllGathers per graph,
                        # each depends only on its 5 Hown tiles so Tile lets
                        # the first half fly mid-loop.
                        W2F = MT * F        # 2560 cols per rank in H
                        HW2 = W2F // 2      # 1280 cols per half
                        for half in range(2):
                            agin = dp.tile([P, HW2], dt.bfloat16,
                                           tag=f"agin{layer}{g}{half}")
                            agout = dp.tile([NC * P, HW2], dt.bfloat16,
                                            tag=f"agout{layer}{g}{half}",
                                            addr_space="Shared")
                            nc.sync.dma_start(
                                agin[:],
                                Hown[:, g * W2F + half * HW2:
                                     g * W2F + (half + 1) * HW2])
                            nc.gpsimd.collective_compute(
                                "AllGather",
                                mybir.AluOpType.bypass,
                                replica_groups=groups,
                                ins=[agin[:].opt()],
                                outs=[agout[:].opt()],
                            )
                            for r in range(NC):
                                # gpsimd queue: reloads must not delay the
                                # next collective's input DMA on the sync ring
                                nc.gpsimd.dma_start(
                                    Hg[g][:, r * W2F + half * HW2:
                                          r * W2F + (half + 1) * HW2],
                                    agout[r * P:(r + 1) * P, :],
                                )
                    elif layer == 1:
                        # AllGather U (half the payload of an H AllGather)
                        WUF = MT * FO       # 1280 cols per rank in U
                        HWU = WUF // 2      # 640 cols per half
                        for half in range(2):
                            agin = dp.tile([P, HWU], dt.bfloat16,
                                           tag=f"aginU{g}{half}")
                            agout = dp.tile([NC * P, HWU], dt.bfloat16,
                                            tag=f"agoutU{g}{half}",
                                            addr_space="Shared")
                            nc.sync.dma_start(
                                agin[:],
                                Uown[:, g * WUF + half * HWU:
                                     g * WUF + (half + 1) * HWU])
                            nc.gpsimd.collective_compute(
                                "AllGather",
                                mybir.AluOpType.bypass,
                                replica_groups=groups,
                                ins=[agin[:].opt()],
                                outs=[agout[:].opt()],
                            )
                            for r in range(NC):
                                nc.gpsimd.dma_start(
                                    Ug[g][:, r * WUF + half * HWU:
                                          r * WUF + (half + 1) * HWU],
                                    agout[r * P:(r + 1) * P, :],
                                )
            nc.sync.dma_start(zout, Zsb[:])
    nc.compile()
    _NC_CACHE["nc"] = nc
    return nc


# ----------------------------------------------------------------------------
# Entry point
# ----------------------------------------------------------------------------

def kernel(x, x_edge_index, y, y_edge_index,
           W1x, b1x, W2x, b2x, W3x, b3x,
           W1y, b1y, W2y, b2y, W3y, b3y,
           _trace=False, _trace_cores=None):
    in_maps = prep_in_maps(x, x_edge_index, y, y_edge_index,
                           W1x, b1x, W2x, b2x, W3x, b3x,
                           W1y, b1y, W2y, b2y, W3y, b3y)
    nc = _build_nc()
    kw = {}
    if _trace:
        kw = dict(trace=True, trace_cores=_trace_cores or [0])
    res = bass_utils.run_bass_kernel_spmd(
        nc, in_maps, core_ids=list(range(NC)), **kw
    )
    z = [res.results[c]["z"] for c in range(NC)]
    out_x = _unshard(z, 0)
    out_y = _unshard(z, 1)
    if _trace:
        kernel._last_result = res
    return out_x, out_y


# revision 4
# speedup vs baseline: 1.4830x; 1.1741x over previous
"""Trainium2 Bass kernel for a 3-layer GCN encoder over two graphs (x, y).

Dense-adjacency formulation:
  GCNConv(h) = D^-1/2 (A+I) D^-1/2 (h @ W) + b
  With Acnt the self-loop-augmented adjacency-count matrix and dinv = deg^-1/2:
      Hhat_1   = dinv * x                   (host, shipped bf16)
      P_l      = Acnt @ Hhat_l              (PE matmul, dominant cost)
      S_l      = dinv * P_l                 (dst-side norm)
      z_l      = S_l @ W_l + b_l            (PE matmul; bias added on vector)
      Hhat_l+1 = dinv * relu(z_l)           (src-side norm of next layer)
  Layer 3 is transform-first: U = Hhat_3 @ W3 is computed on own nodes
  (width 128, not 256), AllGathered, and aggregated as
      out = dinv * (Acnt @ U) + b3
  which halves both the layer-3 aggregation matmul work and the final
  AllGather payload.

Acnt^T is streamed from HBM as fp8e4 (small integer counts, exact).

The TRN2 PE clock ramps to full speed only under ~3us of *continuous*
execution, so the per-m-tile epilogues (transpose / W-GEMM, which wait on
vector-engine copies) are software-pipelined: each m-tile's PE-side
epilogue ops are emitted 1-4 aggregation chains later, keeping the PE
queue stall-free.  Bias adds run on the vector engine (not PE rank-1
matmuls).

Sharding: all 8 cores form one replica group; each core owns a 1280-row
dst shard of BOTH graphs (core 7: 1040 real + 240 pad). Node ids are used
identically (no renumbering); src k-tiles cover rows 0..10111 (KT=79),
the all-pad tail tile is skipped.  One full AllGather per (graph, layer
boundary): graph-major ordering hides each graph's AllGather under the
other graph's aggregation chains.
"""

import numpy as np
import ml_dtypes

import concourse.bass as bass
import concourse.tile as tile
from concourse import bacc, mybir
import concourse.bass_utils as bass_utils
from concourse.masks import make_identity

BF16 = ml_dtypes.bfloat16
FP8 = ml_dtypes.float8_e4m3

P = 128          # partitions / tile edge
NC = 8           # cores
N_NODES = 10000
SHP = 1280       # padded nodes per core (per graph); core 7 has 1040 real
NPAD = NC * SHP  # 10240
KT = 79          # src k-tiles (cover rows 0..10111; tail tile all-pad, skipped)
KTG = 80         # k-tiles in the gathered H/U images (NC*MT; tail never read)
MT = SHP // P    # 10 m-tiles per graph per core
F = 256          # in/hidden feature width
FO = 128         # output feature width

_NC_CACHE = {}


# ----------------------------------------------------------------------------
# Host-side graph preprocessing (index/static work only)
# ----------------------------------------------------------------------------

def _prep_graph(x, edge_index, Ws, bs):
    src = edge_index[0].astype(np.int64)
    dst = edge_index[1].astype(np.int64)
    loop = np.arange(N_NODES, dtype=np.int64)
    src = np.concatenate([src, loop])
    dst = np.concatenate([dst, loop])

    deg = np.zeros(NPAD, np.float32)
    np.add.at(deg, dst, np.float32(1.0))
    dinv = np.zeros(NPAD, np.float32)
    nz = deg > 0
    dinv[nz] = 1.0 / np.sqrt(deg[nz])

    at = np.zeros((KT * P, NPAD), np.float32)   # [src, dst] = A^T counts
    np.add.at(at, (src, dst), np.float32(1.0))

    h1 = np.zeros((KTG * P, F), np.float32)
    h1[:N_NODES] = x * dinv[:N_NODES, None]
    h1_img = np.ascontiguousarray(
        h1.reshape(KTG, P, F).transpose(1, 0, 2).reshape(P, KTG * F)
    ).astype(BF16)

    def w_img(W, fo):
        kf = W.shape[0] // P
        return np.ascontiguousarray(
            W.reshape(kf, P, fo).transpose(1, 0, 2).reshape(P, kf * fo)
        ).astype(BF16)

    slabs = []
    for g in range(NC):
        shard = at[:, g * SHP:(g + 1) * SHP]  # [KT*P src, SHP dst]
        slab = np.ascontiguousarray(
            shard.reshape(KT, P, MT, P).transpose(2, 1, 0, 3).reshape(MT, P, KT * P)
        ).astype(FP8)
        slabs.append(slab)
    w_imgs = [w_img(Ws[0], F), w_img(Ws[1], F), w_img(Ws[2], FO)]

    def bmat(b, fo):
        return np.ascontiguousarray(
            np.tile(np.asarray(b).reshape(1, fo), (P, 1))).astype(np.float32)

    bmats = [bmat(bs[0], F), bmat(bs[1], F), bmat(bs[2], FO)]
    return slabs, h1_img, w_imgs, bmats, dinv


def prep_in_maps(x, x_edge_index, y, y_edge_index,
                 W1x, b1x, W2x, b2x, W3x, b3x,
                 W1y, b1y, W2y, b2y, W3y, b3y):
    sx, h1x, wx, bx, dx = _prep_graph(
        np.asarray(x, np.float32), np.asarray(x_edge_index),
        (np.asarray(W1x), np.asarray(W2x), np.asarray(W3x)),
        (np.asarray(b1x), np.asarray(b2x), np.asarray(b3x)))
    sy, h1y, wy, by, dy = _prep_graph(
        np.asarray(y, np.float32), np.asarray(y_edge_index),
        (np.asarray(W1y), np.asarray(W2y), np.asarray(W3y)),
        (np.asarray(b1y), np.asarray(b2y), np.asarray(b3y)))
    maps = []
    for c in range(NC):
        dvx = dx[c * SHP:(c + 1) * SHP].reshape(MT, P).T   # [P, MT]
        dvy = dy[c * SHP:(c + 1) * SHP].reshape(MT, P).T
        maps.append({
            "at": np.stack([sx[c], sy[c]]),   # [2, MT, P, KT*P] fp8
            "hx": h1x, "hy": h1y,
            "w0": wx[0], "w1": wx[1], "w2": wx[2],
            "w3": wy[0], "w4": wy[1], "w5": wy[2],
            "bm0": bx[0], "bm1": bx[1], "bm2": bx[2],
            "bm3": by[0], "bm4": by[1], "bm5": by[2],
            "dinv": np.ascontiguousarray(
                np.concatenate([dvx, dvy], axis=1)).astype(np.float32),  # [P, 2*MT]
        })
    return maps


def _unshard(z_imgs, graph):
    """8 per-core [P, 2*MT*FO] images -> [N_NODES, FO] for graph 0(x)/1(y)."""
    rows = []
    for c, z in enumerate(z_imgs):
        zi = z.reshape(P, 2 * MT, FO)[:, graph * MT:(graph + 1) * MT, :]
        r = zi.transpose(1, 0, 2).reshape(SHP, FO)
        rows.append(r[:min(SHP, N_NODES - c * SHP)])
    return np.concatenate(rows, axis=0)


# ----------------------------------------------------------------------------
# Device kernel
# ----------------------------------------------------------------------------

def _build_nc():
    if "nc" in _NC_CACHE:
        return _NC_CACHE["nc"]
    nc = bacc.Bacc("TRN2", target_bir_lowering=False, debug=False, num_devices=NC)
    dt = mybir.dt

    at = nc.dram_tensor("at", [2, MT, P, KT * P], dt.float8e4, kind="ExternalInput").ap()
    hx = nc.dram_tensor("hx", [P, KTG * F], dt.bfloat16, kind="ExternalInput").ap()
    hy = nc.dram_tensor("hy", [P, KTG * F], dt.bfloat16, kind="ExternalInput").ap()
    w_ap = [nc.dram_tensor(f"w{i}", [P, 2 * (FO if i % 3 == 2 else F)], dt.bfloat16,
                           kind="ExternalInput").ap() for i in range(6)]
    bm_ap = [nc.dram_tensor(f"bm{i}", [P, FO if i % 3 == 2 else F], dt.float32,
                            kind="ExternalInput").ap() for i in range(6)]
    dinv = nc.dram_tensor("dinv", [P, 2 * MT], dt.float32, kind="ExternalInput").ap()
    zout = nc.dram_tensor("z", [P, 2 * MT * FO], dt.float32, kind="ExternalOutput").ap()

    groups = [list(range(NC))]
    RELU = mybir.ActivationFunctionType.Relu
    ADD = mybir.AluOpType.add

    with tile.TileContext(nc) as tc:
        with (
            tc.tile_pool(name="persist", bufs=1) as pers,
            tc.tile_pool(name="aslab", bufs=3) as apool,
            tc.tile_pool(name="work", bufs=4) as wk,
            tc.tile_pool(name="pagg", bufs=2, space="PSUM") as pagg,
            tc.tile_pool(name="ptr", bufs=2, space="PSUM") as ptr,
            tc.tile_pool(name="pg", bufs=2, space="PSUM") as pg,
            tc.tile_pool(name="dram", bufs=1, space="DRAM") as dp,
        ):
            Hg = [pers.tile([P, KTG * F], dt.bfloat16, name="Hx"),
                  pers.tile([P, KTG * F], dt.bfloat16, name="Hy")]
            Ug = [pers.tile([P, KTG * FO], dt.bfloat16, name="Ux"),
                  pers.tile([P, KTG * FO], dt.bfloat16, name="Uy")]
            Hown = pers.tile([P, 2 * MT * F], dt.bfloat16)
            Uown = pers.tile([P, 2 * MT * FO], dt.bfloat16)
            Zsb = pers.tile([P, 2 * MT * FO], dt.float32)
            Wt = [pers.tile([P, 2 * (FO if i % 3 == 2 else F)], dt.bfloat16,
                            name=f"wt{i}") for i in range(6)]
            Bm = [pers.tile([P, FO if i % 3 == 2 else F], dt.float32,
                            name=f"bm{i}") for i in range(6)]
            Dv = pers.tile([P, 2 * MT], dt.float32)
            ident = pers.tile([P, P], dt.bfloat16)

            make_identity(nc, ident[:])
            # chunked initial H loads so layer-1 matmuls can start early
            nkt = KTG * F
            NCH = 8
            CH = nkt // NCH
            for r in range(NCH):
                nc.sync.dma_start(Hg[0][:, r * CH:(r + 1) * CH],
                                  hx[:, r * CH:(r + 1) * CH])
            for r in range(NCH):
                nc.sync.dma_start(Hg[1][:, r * CH:(r + 1) * CH],
                                  hy[:, r * CH:(r + 1) * CH])
            for i in range(6):
                nc.sync.dma_start(Wt[i][:], w_ap[i])
                nc.sync.dma_start(Bm[i][:], bm_ap[i])
            nc.sync.dma_start(Dv[:], dinv)

            for layer in range(3):
                for g in range(2):
                    Wl = Wt[3 * g + layer]
                    W3l = Wt[3 * g + 2]
                    Bl = Bm[3 * g + layer]
                    B3l = Bm[3 * g + 2]
                    St, STt, GPt, HTt = {}, {}, {}, {}

                    def chain(m, width, img, gm=None):
                        """79-matmul aggregation chain + vector dst-norm."""
                        gm = g * MT + m if gm is None else gm
                        a_slab = apool.tile([P, KT * P], dt.float8e4, tag="aslab")
                        # scalar-engine HWDGE queue: off the sync queue that
                        # carries H/W/B and AG reloads
                        nc.scalar.dma_start(a_slab[:], at[g, m])
                        pP = pagg.tile([P, width], dt.float32, tag="agg")
                        for k in range(KT):
                            nc.tensor.matmul(
                                pP[:],
                                lhsT=a_slab[:, k * P:(k + 1) * P],
                                rhs=img[:, k * width:(k + 1) * width],
                                start=(k == 0),
                                stop=(k == KT - 1),
                            )
                        return pP

                    def emit_S(m, pP):
                        gm = g * MT + m
                        S = wk.tile([P, F], dt.bfloat16, tag="S")
                        nc.vector.tensor_scalar_mul(S[:], pP[:], Dv[:, gm:gm + 1])
                        St[m] = S

                    def emit_TR(j):
                        """PE transposes of S(j); copies drain on vector."""
                        S = St.pop(j)
                        cp = []
                        for kf in range(2):
                            pT = ptr.tile([P, P], dt.bfloat16, tag="tr")
                            nc.tensor.transpose(
                                pT[:], S[:, kf * P:(kf + 1) * P], ident[:])
                            STk = wk.tile([P, P], dt.bfloat16, tag="ST")
                            nc.vector.tensor_copy(STk[:], pT[:])
                            cp.append(STk)
                        STt[j] = cp

                    def emit_WMM(j):
                        """W-GEMM for m-tile j; bias on vector, relu on scalar."""
                        gm = g * MT + j
                        STk = STt.pop(j)
                        gps = pg.tile([P, F], dt.float32, tag="g")
                        for kf in range(2):
                            nc.tensor.matmul(
                                gps[:],
                                lhsT=STk[kf][:],
                                rhs=Wl[:, kf * F:(kf + 1) * F],
                                start=(kf == 0),
                                stop=(kf == 1),
                            )
                        zb = wk.tile([P, F], dt.float32, tag="zb")
                        nc.vector.tensor_tensor(zb[:], gps[:], Bl[:], ADD)
                        # Hhat_{l+1} = dinv * relu(z) for own nodes
                        nc.scalar.activation(
                            Hown[:, gm * F:(gm + 1) * F], zb[:], RELU,
                            scale=Dv[:, gm:gm + 1],
                        )

                    def emit_UTR(j):
                        """PE transposes of Hhat_3(j) for the U = H3 @ W3 GEMM."""
                        gm = g * MT + j
                        cp = []
                        for kf in range(2):
                            pT = ptr.tile([P, P], dt.bfloat16, tag="tr")
                            nc.tensor.transpose(
                                pT[:],
                                Hown[:, gm * F + kf * P:gm * F + (kf + 1) * P],
                                ident[:])
                            HTk = wk.tile([P, P], dt.bfloat16, tag="HT")
                            nc.vector.tensor_copy(HTk[:], pT[:])
                            cp.append(HTk)
                        HTt[j] = cp

                    def emit_UWMM(j):
                        gm = g * MT + j
                        HTk = HTt.pop(j)
                        pU = pg.tile([P, FO], dt.float32, tag="u")
                        for kf in range(2):
                            nc.tensor.matmul(
                                pU[:],
                                lhsT=HTk[kf][:],
                                rhs=W3l[:, kf * FO:(kf + 1) * FO],
                                start=(kf == 0),
                                stop=(kf == 1),
                            )
                        nc.vector.tensor_copy(
                            Uown[:, gm * FO:(gm + 1) * FO], pU[:])

                    if layer < 2:
                        lag = 4 if layer == 1 else 2
                        for t in range(MT + lag):
                            if t < MT:
                                pP = chain(t, F, Hg[g])
                                emit_S(t, pP)
                            if 0 <= t - 1 < MT:
                                emit_TR(t - 1)
                            if 0 <= t - 2 < MT:
                                emit_WMM(t - 2)
                            if layer == 1:
                                if 0 <= t - 3 < MT:
                                    emit_UTR(t - 3)
                                if 0 <= t - 4 < MT:
                                    emit_UWMM(t - 4)
                    else:
                        for t in range(MT):
                            gm = g * MT + t
                            pP = chain(t, FO, Ug[g])
                            # out = dinv * P3 + b3  (vector only; PE stays hot)
                            S3 = wk.tile([P, FO], dt.float32, tag="S3")
                            nc.vector.tensor_scalar_mul(
                                S3[:], pP[:], Dv[:, gm:gm + 1])
                            nc.vector.tensor_tensor(
                                Zsb[:, gm * FO:(gm + 1) * FO], S3[:], B3l[:], ADD)

                    if layer == 0:
                        # One full AllGather of Hhat_2; hides under the other
                        # graph's aggregation chains (graph-major order).
                        W2F = MT * F
                        agin = dp.tile([P, W2F], dt.bfloat16, tag=f"agin{g}")
                        agout = dp.tile([NC * P, W2F], dt.bfloat16,
                                        tag=f"agout{g}", addr_space="Shared")
                        nc.sync.dma_start(
                            agin[:], Hown[:, g * W2F:(g + 1) * W2F])
                        nc.gpsimd.collective_compute(
                            "AllGather",
                            mybir.AluOpType.bypass,
                            replica_groups=groups,
                            ins=[agin[:].opt()],
                            outs=[agout[:].opt()],
                        )
                        for r in range(NC):
                            # gpsimd queue: reloads must not delay the next
                            # collective's input DMA on the sync ring
                            nc.gpsimd.dma_start(
                                Hg[g][:, r * W2F:(r + 1) * W2F],
                                agout[r * P:(r + 1) * P, :],
                            )
                    elif layer == 1:
                        WUF = MT * FO
                        agin = dp.tile([P, WUF], dt.bfloat16, tag=f"aginU{g}")
                        agout = dp.tile([NC * P, WUF], dt.bfloat16,
                                        tag=f"agoutU{g}", addr_space="Shared")
                        nc.sync.dma_start(
                            agin[:], Uown[:, g * WUF:(g + 1) * WUF])
                        nc.gpsimd.collective_compute(
                            "AllGather",
                            mybir.AluOpType.bypass,
                            replica_groups=groups,
                            ins=[agin[:].opt()],
                            outs=[agout[:].opt()],
                        )
                        for r in range(NC):
                            nc.gpsimd.dma_start(
                                Ug[g][:, r * WUF:(r + 1) * WUF],
                                agout[r * P:(r + 1) * P, :],
                            )
            nc.sync.dma_start(zout, Zsb[:])
    nc.compile()
    _NC_CACHE["nc"] = nc
    return nc


# ----------------------------------------------------------------------------
# Entry point
# ----------------------------------------------------------------------------

def kernel(x, x_edge_index, y, y_edge_index,
           W1x, b1x, W2x, b2x, W3x, b3x,
           W1y, b1y, W2y, b2y, W3y, b3y,
           _trace=False, _trace_cores=None):
    in_maps = prep_in_maps(x, x_edge_index, y, y_edge_index,
                           W1x, b1x, W2x, b2x, W3x, b3x,
                           W1y, b1y, W2y, b2y, W3y, b3y)
    nc = _build_nc()
    kw = {}
    if _trace:
        kw = dict(trace=True, trace_cores=_trace_cores or [0])
    res = bass_utils.run_bass_kernel_spmd(
        nc, in_maps, core_ids=list(range(NC)), **kw
    )
    z = [res.results[c]["z"] for c in range(NC)]
    out_x = _unshard(z, 0)
    out_y = _unshard(z, 1)
    if _trace:
        kernel._last_result = res
    return out_x, out_y
